# revision 39
# baseline (speedup 1.0000x reference)
"""Trainium2 Bass kernel for a 2-layer GAT regression model (SPMD, 8 cores).

Model (PyG GATConv semantics, eval mode, self-loops):
  h1 = elu(GATConv(x;   W1, att_src1, att_dst1, b1, heads=4, dim=32))   # concat
  h2 =     GATConv(h1;  W2, att_src2, att_dst2, b2, heads=1, dim=32)
  g  = global_mean_pool(h2, batch);  out = elu(g @ lin1 + b) @ lin2 + b

Distribution: nodes (and their in-edges, by destination) are sharded across
8 cores in 6272-node blocks (49 windows of 128 dst slots each). Edges are
sorted by (dst window, src half); softmax denominators and the weighted
message aggregation are computed per-window in PSUM via matmuls against
host-provided one-hot edge<->slot matrices (fp8, both orientations: p0 for
the aggregation, p0t for the per-edge alpha_dst lookup). Per-edge source
features are fetched with dma_gather from a replicated node table
[h | alpha_src] that each core builds with tensor-engine matmuls against
host-fused weights [W | W@A_src]; rows are 256 B (fp8 for layer 1, fp16 for
layer 2) so each edge costs one tx + one rx DMA descriptor in the SWDGE
ucode -- descriptor generation on GpSimd and the 16 DMA engines' random-read
processing are the bottleneck, so descriptor count is what everything else
is shaped around:
  - self-loops are never gathered: each window's local [h | a_src | a_dst]
    rows stay in SBUF (from the per-window matmuls that also provide the
    alpha_dst table) and the self message is added at finalize time;
  - per (window, half) the gather length is the max real edge count over
    cores (num_idxs_reg), so chunk-rounding padding is never fetched; the
    rotating gather buffers are zeroed once up-front, which keeps the
    never-written pad slots finite (later rotations hold stale rows that
    the all-zero one-hot columns null out exactly).
The node table is split lo/hi (int16 index range); the lo table is built
first, the hi build is emitted as a queue prefix of the lo sweep, and a
20-deep gather pipeline keeps GpSimd desc-gen saturated. Window partials
from the lo sweep accumulate in SBUF and the hi sweep finalizes (softmax
divide + bias + ELU + pooling). Softmax uses no max-shift (logits are
bounded ~[-2, 7] for this model family; exp stays in fp32 range).

Three launches: L1 -> h1 shards; host transpose; L2 -> per-core pooled
partials (graph mean-pool and the layer bias folded in by the host);
host-sum; L3 -> tiny MLP head on one core.
"""

import os
import ml_dtypes
import numpy as np

TRIM = os.environ.get("KERNEL_NOTRIM", "") == ""   # -1 pads + memsets

import concourse.bass as bass
import concourse.mybir as mybir
from concourse import tile, bacc
from concourse.bass_utils import run_bass_kernel_spmd

F32 = mybir.dt.float32
BF16 = mybir.dt.bfloat16
I16 = mybir.dt.int16
F16 = mybir.dt.float16
F8 = mybir.dt.float8e4
AF = mybir.ActivationFunctionType
OP = mybir.AluOpType

N = 50000
E = 800000
FIN = 128
HID = 32
H1 = 4
G = 64
NCORES = 8
SH = 6272                # nodes per shard (49 * 128)
NW = 49                  # dst windows per core
NPAD = SH * NCORES       # 50176 padded node count (= 98 * 512)
HALF = 32768             # int16 gather index limit
NLO = HALF               # lo table rows
NHI = NPAD - HALF        # hi table rows (17408)

last_stats = {}          # test harness introspection: exec times per launch


# ----------------------------------------------------------------------------
# host-side edge structuring
# ----------------------------------------------------------------------------

def _build_edges(edge_index):
    # self-loops are NOT appended: every node's self contribution is computed
    # locally at window-finalize time from SBUF-resident rows (no gather)
    src = np.asarray(edge_index[0]).astype(np.int64)
    dst = np.asarray(edge_index[1]).astype(np.int64)
    core = dst // SH
    ld = dst - core * SH
    win = ld // 128
    rel = ld - win * 128
    half = (src >= HALF).astype(np.int64)

    # per (core, window, half) counts -> uniform chunk counts across cores
    cnt = np.zeros((NCORES, NW, 2), np.int64)
    np.add.at(cnt, (core, win, half), 1)
    CL = np.max(-(-cnt[:, :, 0] // 128), axis=0)     # chunks, lo half
    CH = np.max(-(-cnt[:, :, 1] // 128), axis=0)     # chunks, hi half
    CTW = CL + CH                                    # chunks per window
    woff = np.concatenate([[0], np.cumsum(CTW)])     # chunk offset per window
    CT = int(woff[-1])                               # total chunks
    EP = CT * 128                                    # padded edges per core
    # max real edge count over cores per (win, half): a program-static,
    # core-uniform gather length (num_idxs_reg) -- slots beyond it are never
    # gathered on any core and get memset to zero instead
    maxcnt = np.max(cnt, axis=0)                     # [NW, 2]

    order = np.lexsort((src, half, win, core))
    s_s, s_c, s_w, s_h, s_rel = (a[order] for a in (src, core, win, half, rel))

    # prefix offsets of each (core, win, half) segment in the sorted stream
    seg_of = ((s_c * NW + s_w) * 2 + s_h)
    seg_cnt = np.bincount(seg_of, minlength=NCORES * NW * 2)
    seg_start = np.concatenate([[0], np.cumsum(seg_cnt)])

    f8 = ml_dtypes.float8_e4m3fn
    slots = np.arange(128, dtype=np.float32)
    per_core = []
    for c in range(NCORES):
        src16 = np.zeros(EP, np.int16)               # pad rows gather row 0
        relf = np.full(EP, -1.0, np.float32)
        for w in range(NW):
            base = 128 * int(woff[w])
            for h, nch in ((0, int(CL[w])), (1, int(CH[w]))):
                seg = (c * NW + w) * 2 + h
                a, b = int(seg_start[seg]), int(seg_start[seg + 1])
                L = b - a
                if h == 1:
                    base += 128 * int(CL[w])
                if L:
                    sv = s_s[a:b] - (HALF if h else 0)
                    src16[base:base + L] = sv.astype(np.int16)
                    relf[base:base + L] = s_rel[a:b].astype(np.float32)
        wrap = lambda v: np.tile(np.ascontiguousarray(v.reshape(EP // 16, 16).T), (8, 1))
        relw = relf.reshape(CT, 128)                 # [chunk, edge-in-chunk]
        onehot = (relw[:, :, None] == slots)         # [chunk, edge, slot] bool
        # one interleaved array [slot/edge-major halves] -> one DMA per piece
        pp = np.empty((128, CT, 256), f8)
        pp[:, :, 0:128] = onehot.transpose(2, 0, 1)   # p0t: [slot, chunk, e]
        pp[:, :, 128:256] = onehot.transpose(1, 0, 2)  # p0: [e, chunk, slot]
        per_core.append(dict(srcidx=wrap(src16), p01=np.ascontiguousarray(pp)))
    return per_core, [int(v) for v in CL], [int(v) for v in CH], CT, EP, maxcnt


def _fuse_w(W, a_src, a_dst, heads, dim):
    fin = W.shape[0]
    fo = heads * dim
    As = np.zeros((fo, heads), np.float32)
    Ad = np.zeros((fo, heads), np.float32)
    for h in range(heads):
        As[h * dim:(h + 1) * dim, h] = a_src[h]
        Ad[h * dim:(h + 1) * dim, h] = a_dst[h]
    return np.concatenate([W, W @ As, W @ Ad], axis=1).astype(np.float32)


# ----------------------------------------------------------------------------
# device program
# ----------------------------------------------------------------------------

def _emit_layer(nc, tc, cfg):
    """Emit one GAT layer (+ pooling for layer 2) inside an open TileContext."""
    with tc.tile_pool(name=cfg["tag"] + "stat", bufs=1) as stat:
        _emit_layer_body(nc, tc, cfg, stat)


def _split_pieces(nchk):
    """Split a chunk run into gather pieces of <=8 chunks (1024-idx ucode cap)."""
    out = []
    done = 0
    while done < nchk:
        k = min(8, nchk - done)
        out.append((done, k))
        done += k
    return out


def _emit_layer_body(nc, tc, cfg, stat):
    fo = cfg["fo"]
    heads = cfg["heads"]
    rowlen = cfg["rowlen"]
    tdt = cfg["tdtype"]
    CL, CH = cfg["CL"], cfg["CH"]
    CT, EP = cfg["CT"], cfg["EP"]
    maxcnt = cfg["maxcnt"]
    xT = cfg["xT"]
    xTloc = cfg["xTloc"]
    wext = cfg["wext"]
    brep = cfg["brep"]
    srcidx_d = cfg["srcidx"]
    p01_d = cfg["p01"]
    tbl_lo, tbl_hi = cfg["tbl_lo"], cfg["tbl_hi"]
    wcols = fo + 2 * heads       # [W | W@As | W@Ad] columns
    tcols = fo + heads           # table row payload: [h | alpha_src]
    woff = [0]
    for w in range(NW):
        woff.append(woff[-1] + CL[w] + CH[w])

    wext_sb = stat.tile([128, wcols], BF16)
    nc.sync.dma_start(wext_sb[:], wext[:])
    brep_sb = None
    if cfg["elu"]:
        brep_sb = stat.tile([128, fo], F32)
        nc.sync.dma_start(brep_sb[:], brep[:])
    srcidx_sb = stat.tile([128, EP // 16], I16)
    nc.sync.dma_start(srcidx_sb[:], srcidx_d[:])
    locrow_sb = stat.tile([128, NW, wcols], F16)   # [h | a_src | a_dst] local
    uacc_sb = stat.tile([128, NW, tcols], F32)     # lo-sweep window partials
    if cfg.get("ppool") is not None:
        ppool_sb = stat.tile([128, NW, G], F32)
        nc.sync.dma_start(ppool_sb[:], cfg["ppool"][:])

    xloc_sb = stat.tile([128, SH], BF16)           # whole local shard: 1 DMA
    nc.sync.dma_start(xloc_sb[:], xTloc[:])

    with (
        tc.tile_pool(name=cfg["tag"] + "tb", bufs=7) as tp,
        tc.tile_pool(name=cfg["tag"] + "tbp", bufs=2, space="PSUM") as pp,
        tc.tile_pool(name=cfg["tag"] + "ed", bufs=12) as ep,
        tc.tile_pool(name=cfg["tag"] + "sm", bufs=4) as sp,
        tc.tile_pool(name=cfg["tag"] + "ep", bufs=3, space="PSUM") as pw,
        tc.tile_pool(name=cfg["tag"] + "pp", bufs=1, space="PSUM") as pq,
    ):
        def local_rows(w):
            # [h | a_src | a_dst] of window w's local nodes: feeds the
            # per-edge ad matmuls and the gather-free self-loop contribution.
            # reuse the pt tag: a separate tag would cost 2 more PSUM banks
            pa = pp.tile([128, 2, tcols], F32, tag="pt")
            xl = xloc_sb[:, w * 128:(w + 1) * 128]
            nc.tensor.matmul(pa[:, 0, :], xl, wext_sb[:, 0:tcols],
                             start=True, stop=True)
            nc.tensor.matmul(pa[:, 1, 0:heads], xl, wext_sb[:, tcols:wcols],
                             start=True, stop=True)
            nc.scalar.copy(locrow_sb[:, w, 0:tcols], pa[:, 0, :])
            nc.scalar.copy(locrow_sb[:, w, tcols:wcols], pa[:, 1, 0:heads])

        # --- node table build (two chunks per PSUM tile / copy / write) -----
        def build_iter(r0, tdst, it):
            # one 1024-row iteration; rows [r0+1024*it, ...), tdst rows - r0
            base = r0 + it * 1024
            xt = tp.tile([128, 1024], BF16, tag="xt")
            nc.sync.dma_start(xt[:], xT[:, base:base + 1024])
            for t in range(4):
                pt = pp.tile([128, 2, tcols], F32, tag="pt")
                for u in range(2):
                    nc.tensor.matmul(
                        pt[:, u, :], xt[:, (2 * t + u) * 128:(2 * t + u + 1) * 128],
                        wext_sb[:, 0:tcols], start=True, stop=True)
                sc = tp.tile([128, 2, rowlen], tdt, tag="sc")
                # full-row tile: pad region is uninitialized, never read.
                # alternate ACT/DVE: phase 1 is copy-paced and DVE idles
                if t % 2 == 0:
                    nc.scalar.copy(sc[:, :, 0:tcols], pt[:])
                else:
                    nc.vector.tensor_scalar_add(sc[:, :, 0:tcols], pt[:], 0.0)
                rr = base - r0 + t * 256
                nc.sync.dma_start(
                    tdst[rr:rr + 256, :].rearrange("(c p) e -> p c e", c=2),
                    sc[:])

        # lo table first (its sweep is the long one), local rows woven in;
        # the hi build is emitted as a prefix of the lo sweep: its queue cost
        # drains once up-front while the first gathers are still in flight
        wloc = [0]
        for it in range(NLO // 1024):
            build_iter(0, tbl_lo, it)
            for _ in range(2):
                if wloc[0] < NW:
                    local_rows(wloc[0])
                    wloc[0] += 1
        while wloc[0] < NW:
            local_rows(wloc[0])
            wloc[0] += 1
        for it in range(NHI // 1024):
            build_iter(NLO, tbl_hi, it)

        # --- edge sweeps: all lo pieces, then all hi pieces -----------------
        pool_ps = None
        if cfg.get("ppool") is not None:
            pool_ps = pq.tile([HID, G], F32)
        qctr = 0

        if TRIM:
            # zero all 8 rotating gather buffers once, while DVE is idle.
            # Trimmed gathers skip pad slots; after the first rotation those
            # slots hold stale rows from earlier gathers -- finite values
            # that the all-zero one-hot columns null out exactly, so a
            # one-time zero of the uninitialized SBUF is sufficient.
            for _ in range(20):
                zt = ep.tile([128, 8, rowlen], tdt, tag="gt", bufs=20)
                nc.vector.memset(zt[:], 0)

        def sweep(lo, feed=None, feed_total=0):
            nonlocal qctr
            view = tbl_lo[:, :] if lo else tbl_hi[:, :]
            n_pieces = sum(len(_split_pieces(CL[w] if lo else CH[w]))
                           for w in range(NW))
            fed = piece_i = 0
            for w in range(NW):
                nchk = CL[w] if lo else CH[w]
                seg_gc0 = woff[w] + (0 if lo else CL[w])
                mc = int(maxcnt[w][0 if lo else 1])
                psw = pw.tile([128, tcols], F32, tag="psw")
                adp = pw.tile([128, nchk, heads], F32, tag="pad", bufs=2)
                for (c0, k) in _split_pieces(nchk):
                    if feed is not None:
                        piece_i += 1
                        while fed * n_pieces < feed_total * piece_i:
                            it = next(feed, None)
                            if it is None:
                                break
                            build_iter(0, tbl_lo, it)
                            fed += 1
                    gc = seg_gc0 + c0                 # global chunk index
                    gt = ep.tile([128, k, rowlen], tdt, tag="gt", bufs=20)
                    # zero regions the trimmed gather won't write (padding);
                    # cores with more real edges overwrite via DMA afterwards.
                    # DVE memsets: GpSimd is the desc-gen bottleneck engine.
                    ms = max(mc - c0 * 128, 0)
                    nreg = ms if (TRIM and ms < k * 128) else k * 128
                    nc.gpsimd.dma_gather(
                        gt[:], view, srcidx_sb[:, gc * 8:(gc + k) * 8],
                        k * 128, nreg, rowlen,
                        queue_num=qctr % nc.num_swdge_queues)
                    qctr += 1

                    p01_p = ep.tile([128, k, 256], F8, tag="p01")
                    nc.sync.dma_start(p01_p[:], p01_d[:, gc:gc + k, :])
                    p0t_p = p01_p[:, :, 0:128]
                    p0_p = p01_p[:, :, 128:256]

                    # alpha_dst per edge via matmul vs the local-window table
                    for c in range(k):
                        nc.tensor.matmul(adp[:, c0 + c, :], p0t_p[:, c, :],
                                         locrow_sb[:, w, tcols:wcols],
                                         start=True, stop=True)

                    # alpha = leaky_relu(a_src + a_dst, 0.2); exp (no shift).
                    # PSUM reads on DVE are slow: stage adp through ACT first
                    assb = sp.tile([128, k, heads], F16, tag="assb")
                    nc.scalar.copy(assb[:], gt[:, :, fo:fo + heads])
                    adsb = sp.tile([128, k, heads], F16, tag="adsb")
                    nc.scalar.copy(adsb[:], adp[:, c0:c0 + k, :])
                    al = sp.tile([128, k, heads], F16, tag="al")
                    nc.vector.tensor_add(al[:], assb[:], adsb[:])
                    nc.vector.scalar_tensor_tensor(al[:], al[:], 0.2, al[:],
                                                   OP.mult, OP.max)
                    ex = sp.tile([128, k, heads], F16, tag="ex")
                    nc.scalar.activation(ex[:], al[:], AF.Exp)

                    # messages mt = [h_src * exp | exp]: one moving operand
                    # feeds both aggregation and softmax denominator columns.
                    # ACT writes the strided exp columns: a DVE copy into
                    # stride-tcols singles runs element-at-a-time (~7us/piece)
                    mt = ep.tile([128, k, tcols], F16, tag="mt", bufs=10)
                    nc.scalar.activation(mt[:, :, fo:fo + heads], al[:], AF.Exp)
                    nc.vector.tensor_tensor(
                        mt[:, :, 0:fo].rearrange("p k (h d) -> p k h d", d=HID),
                        gt[:, :, 0:fo].rearrange("p k (h d) -> p k h d", d=HID),
                        ex[:].unsqueeze(3).broadcast_to([128, k, heads, HID]),
                        OP.mult)

                    # window aggregation in PSUM: [agg | denom] per chunk
                    for c in range(k):
                        nc.tensor.matmul(psw[:], p0_p[:, c, :], mt[:, c, :],
                                         start=(c0 + c == 0),
                                         stop=(c0 + c == nchk - 1))

                if lo:
                    # save the window partial; the hi sweep finishes it
                    nc.scalar.copy(uacc_sb[:, w, :], psw[:])
                    continue

                # final sweep: combine halves + the local self-loop message,
                # out = agg/denom + bias.
                # +eps matches the reference and keeps empty (padding) slots
                # at exactly 0 instead of 0*inf=NaN, which would poison the
                # pool matmul through its contraction over node partitions.
                als = sp.tile([128, heads], F16, tag="als")
                nc.vector.tensor_add(als[:], locrow_sb[:, w, fo:fo + heads],
                                     locrow_sb[:, w, tcols:wcols])
                nc.vector.scalar_tensor_tensor(als[:], als[:], 0.2, als[:],
                                               OP.mult, OP.max)
                exs = sp.tile([128, heads], F32, tag="exs")
                nc.scalar.activation(exs[:], als[:], AF.Exp)
                pswsb = sp.tile([128, tcols], F32, tag="pswsb")
                nc.scalar.copy(pswsb[:], psw[:])
                usb = sp.tile([128, tcols], F32, tag="usb")
                nc.vector.tensor_add(usb[:], uacc_sb[:, w, :], pswsb[:])
                # self message: num += h_loc * exp(alpha_self), den += exp
                nc.vector.tensor_tensor(
                    pswsb[:, 0:fo].rearrange("p (h d) -> p h d", d=HID),
                    locrow_sb[:, w, 0:fo].rearrange("p (h d) -> p h d", d=HID),
                    exs[:].unsqueeze(2).broadcast_to([128, heads, HID]),
                    OP.mult)
                nc.vector.tensor_add(usb[:, 0:fo], usb[:, 0:fo], pswsb[:, 0:fo])
                nc.vector.tensor_add(usb[:, fo:fo + heads],
                                     usb[:, fo:fo + heads], exs[:])
                rc = sp.tile([128, heads], F32, tag="rc")
                nc.vector.tensor_scalar_add(rc[:], usb[:, fo:fo + heads], 1e-16)
                nc.vector.reciprocal(rc[:], rc[:])
                u = sp.tile([128, fo], F32, tag="u")
                nc.vector.tensor_tensor(
                    u[:].rearrange("p (h d) -> p h d", d=HID),
                    usb[:, 0:fo].rearrange("p (h d) -> p h d", d=HID),
                    rc[:].unsqueeze(2).broadcast_to([128, heads, HID]),
                    OP.mult)
                if cfg["elu"]:
                    nc.vector.tensor_add(u[:], u[:], brep_sb[:])
                    r1 = sp.tile([128, fo], F32, tag="r1")
                    nc.scalar.activation(r1[:], u[:], AF.Relu)
                    r2 = sp.tile([128, fo], F32, tag="r2")
                    nc.scalar.activation(r2[:], u[:], AF.Relu, scale=-1.0)
                    e2 = sp.tile([128, fo], F32, tag="e2")
                    nc.scalar.activation(e2[:], r2[:], AF.Exp, scale=-1.0)
                    nc.vector.scalar_tensor_tensor(u[:], e2[:], -1.0, r1[:],
                                                   OP.add, OP.add)
                if cfg.get("hout") is not None:
                    nc.sync.dma_start(cfg["hout"][w * 128:(w + 1) * 128, :], u[:])
                if pool_ps is not None:
                    nc.tensor.matmul(pool_ps[:], u[:], ppool_sb[:, w, :],
                                     start=(w == 0), stop=(w == NW - 1))

        sweep(lo=True)
        sweep(lo=False)

        if pool_ps is not None:
            po = sp.tile([HID, G], F32, tag="po")
            nc.scalar.copy(po[:], pool_ps[:])
            nc.sync.dma_start(cfg["poolout"][:], po[:])


def _build_layer_program(cfg):
    nc = bacc.Bacc(None, target_bir_lowering=False, num_swdge_queues=4)
    d = {}
    d["xT"] = nc.declare_dram_parameter("xT", [128, NPAD], BF16, isOutput=False)
    d["xTloc"] = nc.declare_dram_parameter("xTloc", [128, SH], BF16, isOutput=False)
    wcols = cfg["fo"] + 2 * cfg["heads"]
    d["wext"] = nc.declare_dram_parameter("wext", [128, wcols], BF16, isOutput=False)
    d["brep"] = nc.declare_dram_parameter("brep", [128, cfg["fo"]], F32, isOutput=False)
    d["srcidx"] = nc.declare_dram_parameter("srcidx", [128, cfg["EP"] // 16], I16, isOutput=False)
    d["tbl_lo"] = nc.dram_tensor("tbl_lo", [NLO, cfg["rowlen"]], cfg["tdtype"])
    d["tbl_hi"] = nc.dram_tensor("tbl_hi", [NHI, cfg["rowlen"]], cfg["tdtype"])
    d["p01"] = nc.declare_dram_parameter("p01", [128, cfg["CT"], 256], F8, isOutput=False)
    if cfg["pool"]:
        d["ppool"] = nc.declare_dram_parameter("ppool", [128, NW, G], F32, isOutput=False)
        d["poolout"] = nc.declare_dram_parameter("poolout", [HID, G], F32, isOutput=True)
    else:
        d["hout"] = nc.declare_dram_parameter("hout", [SH, cfg["fo"]], F32, isOutput=True)
    cfg = dict(cfg, **d)
    with tile.TileContext(nc) as tc:
        _emit_layer(nc, tc, cfg)
    nc.compile()
    return nc


def _build_head_program():
    nc = bacc.Bacc(None, target_bir_lowering=False)
    poolT = nc.declare_dram_parameter("poolT", [HID, G], F32, isOutput=False)
    l1w = nc.declare_dram_parameter("l1w", [HID, HID // 2], F32, isOutput=False)
    l1b = nc.declare_dram_parameter("l1b", [HID // 2, 1], F32, isOutput=False)
    l1bn = nc.declare_dram_parameter("l1bn", [HID // 2, 1], F32, isOutput=False)
    l2w = nc.declare_dram_parameter("l2w", [HID // 2, 1], F32, isOutput=False)
    l2b = nc.declare_dram_parameter("l2b", [1, 1], F32, isOutput=False)
    outT = nc.declare_dram_parameter("outT", [1, G], F32, isOutput=True)
    with tile.TileContext(nc) as tc:
        with (
            tc.tile_pool(name="h", bufs=1) as hp,
            tc.tile_pool(name="hp", bufs=2, space="PSUM") as pp,
        ):
            pt = hp.tile([HID, G], F32)
            nc.sync.dma_start(pt[:], poolT[:])
            w1 = hp.tile([HID, HID // 2], F32)
            nc.sync.dma_start(w1[:], l1w[:])
            b1 = hp.tile([HID // 2, 1], F32)
            nc.sync.dma_start(b1[:], l1b[:])
            b1n = hp.tile([HID // 2, 1], F32)
            nc.sync.dma_start(b1n[:], l1bn[:])
            w2 = hp.tile([HID // 2, 1], F32)
            nc.sync.dma_start(w2[:], l2w[:])
            b2 = hp.tile([1, 1], F32)
            nc.sync.dma_start(b2[:], l2b[:])

            ps1 = pp.tile([HID // 2, G], F32)
            nc.tensor.matmul(ps1[:], w1[:], pt[:], start=True, stop=True)
            r1 = hp.tile([HID // 2, G], F32)
            nc.scalar.activation(r1[:], ps1[:], AF.Relu, bias=b1[:, 0:1])
            r2 = hp.tile([HID // 2, G], F32)
            nc.scalar.activation(r2[:], ps1[:], AF.Relu, scale=-1.0, bias=b1n[:, 0:1])
            e2 = hp.tile([HID // 2, G], F32)
            nc.scalar.activation(e2[:], r2[:], AF.Exp, scale=-1.0)
            h = hp.tile([HID // 2, G], F32)
            nc.vector.tensor_add(h[:], r1[:], e2[:])
            nc.vector.tensor_scalar_add(h[:], h[:], -1.0)
            ps2 = pp.tile([1, G], F32)
            nc.tensor.matmul(ps2[:], w2[:], h[:], start=True, stop=True)
            o = hp.tile([1, G], F32)
            nc.scalar.activation(o[:], ps2[:], AF.Identity, bias=b2[0:1, 0:1])
            nc.sync.dma_start(outT[:], o[:])
    nc.compile()
    return nc


# ----------------------------------------------------------------------------
# entry point
# ----------------------------------------------------------------------------

def _run(nc, in_maps, core_ids, trace=False):
    import os
    tr = trace or bool(os.environ.get("KERNEL_PROFILE"))
    res = run_bass_kernel_spmd(nc, in_maps, core_ids, trace=tr)
    if res.exec_time_ns is not None:
        last_stats.setdefault("exec_ns", []).append(res.exec_time_ns)
    return res


def kernel(x, edge_index, batch, W1, att_src1, att_dst1, b1, W2, att_src2,
           att_dst2, b2, lin1_w, lin1_b, lin2_w, lin2_b):
    x = np.asarray(x, np.float32)
    per_core, CL, CH, CT, EP, maxcnt = _build_edges(edge_index)
    batch = np.asarray(batch).astype(np.int64)

    bf = ml_dtypes.bfloat16
    w1ext = _fuse_w(np.asarray(W1, np.float32),
                    np.asarray(att_src1, np.float32), np.asarray(att_dst1, np.float32),
                    H1, HID)
    w2ext = _fuse_w(np.asarray(W2, np.float32),
                    np.asarray(att_src2, np.float32), np.asarray(att_dst2, np.float32),
                    1, HID)
    b1rep = np.tile(np.asarray(b1, np.float32), (128, 1)).copy()
    b2rep = np.tile(np.asarray(b2, np.float32), (128, 1)).copy()

    xp = np.zeros((NPAD, FIN), np.float32)
    xp[:N] = x
    xT = np.ascontiguousarray(xp.T.astype(bf))

    base_cfg = dict(CL=CL, CH=CH, CT=CT, EP=EP, maxcnt=maxcnt)
    cfg1 = dict(base_cfg, tag="a", fo=FIN, heads=H1, rowlen=256, tdtype=F8,
                elu=True, pool=False)
    cfg2 = dict(base_cfg, tag="b", fo=HID, heads=1, rowlen=128, tdtype=F16,
                elu=False, pool=True)

    nc1 = _build_layer_program(cfg1)
    in_maps = []
    for c in range(NCORES):
        in_maps.append(dict(
            xT=xT, xTloc=np.ascontiguousarray(xT[:, c * SH:(c + 1) * SH]),
            wext=w1ext.astype(bf), brep=b1rep, **per_core[c]))
    r1 = _run(nc1, in_maps, list(range(NCORES)))

    h1 = np.zeros((NPAD, FIN), np.float32)
    for c in range(NCORES):
        lo, hi = c * SH, min((c + 1) * SH, N)
        h1[lo:hi] = r1.results[c]["hout"][:hi - lo]
    h1T = np.ascontiguousarray(h1.T.astype(bf))

    # pooling matrices with 1/count folded in
    counts = np.bincount(batch, minlength=G).astype(np.float32)
    recip = 1.0 / np.maximum(counts, 1.0)
    nc2 = _build_layer_program(cfg2)
    in_maps2 = []
    for c in range(NCORES):
        pb = np.zeros((SH, G), np.float32)
        lo, hi = c * SH, min((c + 1) * SH, N)
        if hi > lo:
            rows = np.arange(hi - lo)
            pb[rows, batch[lo:hi]] = recip[batch[lo:hi]]
        ppool = np.ascontiguousarray(pb.reshape(NW, 128, G).transpose(1, 0, 2))
        in_maps2.append(dict(
            xT=h1T, xTloc=np.ascontiguousarray(h1T[:, c * SH:(c + 1) * SH]),
            wext=w2ext.astype(bf), brep=b2rep, ppool=ppool, **per_core[c]))
    r2 = _run(nc2, in_maps2, list(range(NCORES)))

    poolT = np.zeros((HID, G), np.float32)
    for c in range(NCORES):
        poolT += r2.results[c]["poolout"]
    # layer-2 bias is linear through the mean-pool: add it here instead of
    # per-node on device
    poolT += np.asarray(b2, np.float32).reshape(HID, 1)

    nc3 = _build_head_program()
    l1b_ = np.asarray(lin1_b, np.float32).reshape(HID // 2, 1)
    r3 = _run(nc3, [dict(
        poolT=poolT, l1w=np.asarray(lin1_w, np.float32), l1b=l1b_, l1bn=-l1b_,
        l2w=np.asarray(lin2_w, np.float32),
        l2b=np.asarray(lin2_b, np.float32).reshape(1, 1))], [0])
    return np.ascontiguousarray(r3.results[0]["outT"].reshape(G, 1))


# revision 40
# speedup vs baseline: 1.0660x; 1.0660x over previous
"""Trainium2 Bass kernel for a 2-layer GAT regression model (SPMD, 8 cores).

Model (PyG GATConv semantics, eval mode, self-loops):
  h1 = elu(GATConv(x;   W1, att_src1, att_dst1, b1, heads=4, dim=32))   # concat
  h2 =     GATConv(h1;  W2, att_src2, att_dst2, b2, heads=1, dim=32)
  g  = global_mean_pool(h2, batch);  out = elu(g @ lin1 + b) @ lin2 + b

Distribution: nodes (and their in-edges, by destination) are sharded across
8 cores in 6272-node blocks (49 windows of 128 dst slots each). Edges are
sorted by (dst window, src half); softmax denominators and the weighted
message aggregation are computed per-window in PSUM via matmuls against
host-provided one-hot edge<->slot matrices (fp8, both orientations: p0 for
the aggregation, p0t for the per-edge alpha_dst lookup). Per-edge source
features are fetched with dma_gather from a replicated node table
[h | alpha_src] that each core builds with tensor-engine matmuls against
host-fused weights [W | W@A_src]; rows are 256 B (fp8 for layer 1, fp16 for
layer 2) so each edge costs one tx + one rx DMA descriptor in the SWDGE
ucode -- descriptor generation on GpSimd and the 16 DMA engines' random-read
processing are the bottleneck, so descriptor count is what everything else
is shaped around:
  - self-loops are never gathered: each window's local [h | a_src | a_dst]
    rows stay in SBUF (from the per-window matmuls that also provide the
    alpha_dst table) and the self message is added at finalize time;
  - per (window, half) the gather length is the max real edge count over
    cores (num_idxs_reg), so chunk-rounding padding is never fetched; the
    rotating gather buffers are zeroed once up-front, which keeps the
    never-written pad slots finite (later rotations hold stale rows that
    the all-zero one-hot columns null out exactly).
The node table is split lo/hi (int16 index range); the lo table is built
first, the hi build is emitted as a queue prefix of the lo sweep, and a
20-deep gather pipeline keeps GpSimd desc-gen saturated. Window partials
from the lo sweep accumulate in SBUF and the hi sweep finalizes (softmax
divide + bias + ELU + pooling). Softmax uses no max-shift (logits are
bounded ~[-2, 7] for this model family; exp stays in fp32 range).

Three launches: L1 -> h1 shards; host transpose; L2 -> per-core pooled
partials (graph mean-pool and the layer bias folded in by the host);
host-sum; L3 -> tiny MLP head on one core.
"""

import os
import ml_dtypes
import numpy as np

TRIM = os.environ.get("KERNEL_NOTRIM", "") == ""   # -1 pads + memsets

import concourse.bass as bass
import concourse.mybir as mybir
from concourse import tile, bacc
from concourse.bass_utils import run_bass_kernel_spmd

F32 = mybir.dt.float32
BF16 = mybir.dt.bfloat16
I16 = mybir.dt.int16
F16 = mybir.dt.float16
F8 = mybir.dt.float8e4
AF = mybir.ActivationFunctionType
OP = mybir.AluOpType

N = 50000
E = 800000
FIN = 128
HID = 32
H1 = 4
G = 64
NCORES = 8
SH = 6272                # nodes per shard (49 * 128)
NW = 49                  # dst windows per core
NPAD = SH * NCORES       # 50176 padded node count (= 98 * 512)
HALF = 32768             # int16 gather index limit
NLO = HALF               # lo table rows
NHI = NPAD - HALF        # hi table rows (17408)

last_stats = {}          # test harness introspection: exec times per launch


# ----------------------------------------------------------------------------
# host-side edge structuring
# ----------------------------------------------------------------------------

def _build_edges(edge_index):
    # self-loops are NOT appended: every node's self contribution is computed
    # locally at window-finalize time from SBUF-resident rows (no gather)
    src = np.asarray(edge_index[0]).astype(np.int64)
    dst = np.asarray(edge_index[1]).astype(np.int64)
    core = dst // SH
    ld = dst - core * SH
    win = ld // 128
    rel = ld - win * 128
    half = (src >= HALF).astype(np.int64)

    # per (core, window, half) counts -> uniform chunk counts across cores
    cnt = np.zeros((NCORES, NW, 2), np.int64)
    np.add.at(cnt, (core, win, half), 1)
    CL = np.max(-(-cnt[:, :, 0] // 128), axis=0)     # chunks, lo half
    CH = np.max(-(-cnt[:, :, 1] // 128), axis=0)     # chunks, hi half
    CTW = CL + CH                                    # chunks per window
    woff = np.concatenate([[0], np.cumsum(CTW)])     # chunk offset per window
    CT = int(woff[-1])                               # total chunks
    EP = CT * 128                                    # padded edges per core
    # max real edge count over cores per (win, half): a program-static,
    # core-uniform gather length (num_idxs_reg) -- slots beyond it are never
    # gathered on any core and get memset to zero instead
    maxcnt = np.max(cnt, axis=0)                     # [NW, 2]

    order = np.lexsort((src, half, win, core))
    s_s, s_c, s_w, s_h, s_rel = (a[order] for a in (src, core, win, half, rel))

    # prefix offsets of each (core, win, half) segment in the sorted stream
    seg_of = ((s_c * NW + s_w) * 2 + s_h)
    seg_cnt = np.bincount(seg_of, minlength=NCORES * NW * 2)
    seg_start = np.concatenate([[0], np.cumsum(seg_cnt)])

    f8 = ml_dtypes.float8_e4m3fn
    slots = np.arange(128, dtype=np.float32)
    per_core = []
    for c in range(NCORES):
        src16 = np.zeros(EP, np.int16)               # pad rows gather row 0
        relf = np.full(EP, -1.0, np.float32)
        for w in range(NW):
            base = 128 * int(woff[w])
            for h, nch in ((0, int(CL[w])), (1, int(CH[w]))):
                seg = (c * NW + w) * 2 + h
                a, b = int(seg_start[seg]), int(seg_start[seg + 1])
                L = b - a
                if h == 1:
                    base += 128 * int(CL[w])
                if L:
                    sv = s_s[a:b] - (HALF if h else 0)
                    src16[base:base + L] = sv.astype(np.int16)
                    relf[base:base + L] = s_rel[a:b].astype(np.float32)
        wrap = lambda v: np.tile(np.ascontiguousarray(v.reshape(EP // 16, 16).T), (8, 1))
        relw = relf.reshape(CT, 128)                 # [chunk, edge-in-chunk]
        onehot = (relw[:, :, None] == slots)         # [chunk, edge, slot] bool
        # one interleaved array [slot/edge-major halves] -> one DMA per piece
        pp = np.empty((128, CT, 256), f8)
        pp[:, :, 0:128] = onehot.transpose(2, 0, 1)   # p0t: [slot, chunk, e]
        pp[:, :, 128:256] = onehot.transpose(1, 0, 2)  # p0: [e, chunk, slot]
        per_core.append(dict(srcidx=wrap(src16), p01=np.ascontiguousarray(pp)))
    return per_core, [int(v) for v in CL], [int(v) for v in CH], CT, EP, maxcnt


def _fuse_w(W, a_src, a_dst, heads, dim):
    fin = W.shape[0]
    fo = heads * dim
    As = np.zeros((fo, heads), np.float32)
    Ad = np.zeros((fo, heads), np.float32)
    for h in range(heads):
        As[h * dim:(h + 1) * dim, h] = a_src[h]
        Ad[h * dim:(h + 1) * dim, h] = a_dst[h]
    return np.concatenate([W, W @ As, W @ Ad], axis=1).astype(np.float32)


# ----------------------------------------------------------------------------
# device program
# ----------------------------------------------------------------------------

def _emit_layer(nc, tc, cfg):
    """Emit one GAT layer (+ pooling for layer 2) inside an open TileContext."""
    with tc.tile_pool(name=cfg["tag"] + "stat", bufs=1) as stat:
        _emit_layer_body(nc, tc, cfg, stat)


def _split_pieces(nchk):
    """Split a chunk run into gather pieces of <=8 chunks (1024-idx ucode cap)."""
    out = []
    done = 0
    while done < nchk:
        k = min(8, nchk - done)
        out.append((done, k))
        done += k
    return out


def _emit_layer_body(nc, tc, cfg, stat):
    fo = cfg["fo"]
    heads = cfg["heads"]
    rowlen = cfg["rowlen"]
    tdt = cfg["tdtype"]
    CL, CH = cfg["CL"], cfg["CH"]
    CT, EP = cfg["CT"], cfg["EP"]
    maxcnt = cfg["maxcnt"]
    xT = cfg["xT"]
    xTloc = cfg["xTloc"]
    wext = cfg["wext"]
    brep = cfg["brep"]
    srcidx_d = cfg["srcidx"]
    p01_d = cfg["p01"]
    tbl_lo, tbl_hi = cfg["tbl_lo"], cfg["tbl_hi"]
    wcols = fo + 2 * heads       # [W | W@As | W@Ad] columns
    tcols = fo + heads           # table row payload: [h | alpha_src]
    woff = [0]
    for w in range(NW):
        woff.append(woff[-1] + CL[w] + CH[w])

    wext_sb = stat.tile([128, wcols], BF16)
    nc.sync.dma_start(wext_sb[:], wext[:])
    brep_sb = None
    if cfg["elu"]:
        brep_sb = stat.tile([128, fo], F32)
        nc.sync.dma_start(brep_sb[:], brep[:])
    srcidx_sb = stat.tile([128, EP // 16], I16)
    nc.sync.dma_start(srcidx_sb[:], srcidx_d[:])
    locrow_sb = stat.tile([128, NW, wcols], F16)   # [h | a_src | a_dst] local
    uacc_sb = stat.tile([128, NW, tcols], F32)     # lo-sweep window partials
    if cfg.get("ppool") is not None:
        ppool_sb = stat.tile([128, NW, G], F32)
        nc.sync.dma_start(ppool_sb[:], cfg["ppool"][:])

    xloc_sb = stat.tile([128, SH], BF16)           # whole local shard: 1 DMA
    nc.sync.dma_start(xloc_sb[:], xTloc[:])

    with (
        tc.tile_pool(name=cfg["tag"] + "tb", bufs=5) as tp,
        tc.tile_pool(name=cfg["tag"] + "tbp", bufs=2, space="PSUM") as pp,
        tc.tile_pool(name=cfg["tag"] + "ed", bufs=12) as ep,
        tc.tile_pool(name=cfg["tag"] + "sm", bufs=4) as sp,
        tc.tile_pool(name=cfg["tag"] + "ep", bufs=3, space="PSUM") as pw,
        tc.tile_pool(name=cfg["tag"] + "pp", bufs=1, space="PSUM") as pq,
    ):
        def local_rows(w):
            # [h | a_src | a_dst] of window w's local nodes: feeds the
            # per-edge ad matmuls and the gather-free self-loop contribution.
            # reuse the pt tag: a separate tag would cost 2 more PSUM banks
            pa = pp.tile([128, 2, tcols], F32, tag="pt")
            xl = xloc_sb[:, w * 128:(w + 1) * 128]
            nc.tensor.matmul(pa[:, 0, :], xl, wext_sb[:, 0:tcols],
                             start=True, stop=True)
            nc.tensor.matmul(pa[:, 1, 0:heads], xl, wext_sb[:, tcols:wcols],
                             start=True, stop=True)
            nc.scalar.copy(locrow_sb[:, w, 0:tcols], pa[:, 0, :])
            nc.scalar.copy(locrow_sb[:, w, tcols:wcols], pa[:, 1, 0:heads])

        # --- node table build (two chunks per PSUM tile / copy / write) -----
        def build_iter(r0, tdst, it):
            # one 1024-row iteration; rows [r0+1024*it, ...), tdst rows - r0
            base = r0 + it * 1024
            xt = tp.tile([128, 1024], BF16, tag="xt")
            nc.sync.dma_start(xt[:], xT[:, base:base + 1024])
            for t in range(4):
                pt = pp.tile([128, 2, tcols], F32, tag="pt")
                for u in range(2):
                    nc.tensor.matmul(
                        pt[:, u, :], xt[:, (2 * t + u) * 128:(2 * t + u + 1) * 128],
                        wext_sb[:, 0:tcols], start=True, stop=True)
                sc = tp.tile([128, 2, rowlen], tdt, tag="sc")
                # full-row tile: pad region is uninitialized, never read.
                # alternate ACT/DVE: phase 1 is copy-paced and DVE idles
                if t % 2 == 0:
                    nc.scalar.copy(sc[:, :, 0:tcols], pt[:])
                else:
                    nc.vector.tensor_scalar_add(sc[:, :, 0:tcols], pt[:], 0.0)
                rr = base - r0 + t * 256
                nc.sync.dma_start(
                    tdst[rr:rr + 256, :].rearrange("(c p) e -> p c e", c=2),
                    sc[:])

        # lo table first (its sweep is the long one), local rows woven in;
        # the hi build is emitted as a prefix of the lo sweep: its queue cost
        # drains once up-front while the first gathers are still in flight
        wloc = [0]
        for it in range(NLO // 1024):
            build_iter(0, tbl_lo, it)
            for _ in range(2):
                if wloc[0] < NW:
                    local_rows(wloc[0])
                    wloc[0] += 1
        while wloc[0] < NW:
            local_rows(wloc[0])
            wloc[0] += 1
        for it in range(NHI // 1024):
            build_iter(NLO, tbl_hi, it)

        # --- edge sweeps: all lo pieces, then all hi pieces -----------------
        pool_ps = None
        if cfg.get("ppool") is not None:
            pool_ps = pq.tile([HID, G], F32)
        qctr = 0

        if TRIM:
            # zero all 8 rotating gather buffers once, while DVE is idle.
            # Trimmed gathers skip pad slots; after the first rotation those
            # slots hold stale rows from earlier gathers -- finite values
            # that the all-zero one-hot columns null out exactly, so a
            # one-time zero of the uninitialized SBUF is sufficient.
            for _ in range(20):
                zt = ep.tile([128, 8, rowlen], tdt, tag="gt", bufs=20)
                nc.vector.memset(zt[:], 0)

        def sweep(lo, feed=None, feed_total=0):
            nonlocal qctr
            view = tbl_lo[:, :] if lo else tbl_hi[:, :]
            n_pieces = sum(len(_split_pieces(CL[w] if lo else CH[w]))
                           for w in range(NW))
            fed = piece_i = 0
            for w in range(NW):
                nchk = CL[w] if lo else CH[w]
                seg_gc0 = woff[w] + (0 if lo else CL[w])
                mc = int(maxcnt[w][0 if lo else 1])
                psw = pw.tile([128, tcols], F32, tag="psw")
                adp = pw.tile([128, nchk, heads], F32, tag="pad", bufs=2)
                for (c0, k) in _split_pieces(nchk):
                    if feed is not None:
                        piece_i += 1
                        while fed * n_pieces < feed_total * piece_i:
                            it = next(feed, None)
                            if it is None:
                                break
                            build_iter(0, tbl_lo, it)
                            fed += 1
                    gc = seg_gc0 + c0                 # global chunk index
                    gt = ep.tile([128, k, rowlen], tdt, tag="gt", bufs=20)
                    # zero regions the trimmed gather won't write (padding);
                    # cores with more real edges overwrite via DMA afterwards.
                    # DVE memsets: GpSimd is the desc-gen bottleneck engine.
                    ms = max(mc - c0 * 128, 0)
                    nreg = ms if (TRIM and ms < k * 128) else k * 128
                    nc.gpsimd.dma_gather(
                        gt[:], view, srcidx_sb[:, gc * 8:(gc + k) * 8],
                        k * 128, nreg, rowlen,
                        queue_num=qctr % nc.num_swdge_queues)
                    qctr += 1

                    p01_p = ep.tile([128, k, 256], F8, tag="p01")
                    nc.sync.dma_start(p01_p[:], p01_d[:, gc:gc + k, :])
                    p0t_p = p01_p[:, :, 0:128]
                    p0_p = p01_p[:, :, 128:256]

                    # alpha_dst per edge via matmul vs the local-window table
                    for c in range(k):
                        nc.tensor.matmul(adp[:, c0 + c, :], p0t_p[:, c, :],
                                         locrow_sb[:, w, tcols:wcols],
                                         start=True, stop=True)

                    # alpha = leaky_relu(a_src + a_dst, 0.2); exp (no shift).
                    # PSUM reads on DVE are slow: stage adp through ACT first
                    assb = sp.tile([128, k, heads], F16, tag="assb")
                    nc.scalar.copy(assb[:], gt[:, :, fo:fo + heads])
                    adsb = sp.tile([128, k, heads], F16, tag="adsb")
                    nc.scalar.copy(adsb[:], adp[:, c0:c0 + k, :])
                    al = sp.tile([128, k, heads], F16, tag="al")
                    nc.vector.tensor_add(al[:], assb[:], adsb[:])
                    nc.vector.scalar_tensor_tensor(al[:], al[:], 0.2, al[:],
                                                   OP.mult, OP.max)
                    ex = sp.tile([128, k, heads], F16, tag="ex")
                    nc.scalar.activation(ex[:], al[:], AF.Exp)

                    # messages mt = [h_src * exp | exp]: one moving operand
                    # feeds both aggregation and softmax denominator columns.
                    # ACT writes the strided exp columns: a DVE copy into
                    # stride-tcols singles runs element-at-a-time (~7us/piece)
                    mt = ep.tile([128, k, tcols], F16, tag="mt", bufs=10)
                    nc.scalar.activation(mt[:, :, fo:fo + heads], al[:], AF.Exp)
                    nc.vector.tensor_tensor(
                        mt[:, :, 0:fo].rearrange("p k (h d) -> p k h d", d=HID),
                        gt[:, :, 0:fo].rearrange("p k (h d) -> p k h d", d=HID),
                        ex[:].unsqueeze(3).broadcast_to([128, k, heads, HID]),
                        OP.mult)

                    # window aggregation in PSUM: [agg | denom] per chunk
                    for c in range(k):
                        nc.tensor.matmul(psw[:], p0_p[:, c, :], mt[:, c, :],
                                         start=(c0 + c == 0),
                                         stop=(c0 + c == nchk - 1))

                if lo:
                    # save the window partial; the hi sweep finishes it
                    nc.scalar.copy(uacc_sb[:, w, :], psw[:])
                    continue

                # final sweep: combine halves + the local self-loop message,
                # out = agg/denom + bias.
                # +eps matches the reference and keeps empty (padding) slots
                # at exactly 0 instead of 0*inf=NaN, which would poison the
                # pool matmul through its contraction over node partitions.
                als = sp.tile([128, heads], F16, tag="als")
                nc.vector.tensor_add(als[:], locrow_sb[:, w, fo:fo + heads],
                                     locrow_sb[:, w, tcols:wcols])
                nc.vector.scalar_tensor_tensor(als[:], als[:], 0.2, als[:],
                                               OP.mult, OP.max)
                exs = sp.tile([128, heads], F32, tag="exs")
                nc.scalar.activation(exs[:], als[:], AF.Exp)
                pswsb = sp.tile([128, tcols], F32, tag="pswsb")
                nc.scalar.copy(pswsb[:], psw[:])
                usb = sp.tile([128, tcols], F32, tag="usb")
                nc.vector.tensor_add(usb[:], uacc_sb[:, w, :], pswsb[:])
                # self message: num += h_loc * exp(alpha_self), den += exp
                nc.vector.tensor_tensor(
                    pswsb[:, 0:fo].rearrange("p (h d) -> p h d", d=HID),
                    locrow_sb[:, w, 0:fo].rearrange("p (h d) -> p h d", d=HID),
                    exs[:].unsqueeze(2).broadcast_to([128, heads, HID]),
                    OP.mult)
                nc.vector.tensor_add(usb[:, 0:fo], usb[:, 0:fo], pswsb[:, 0:fo])
                nc.vector.tensor_add(usb[:, fo:fo + heads],
                                     usb[:, fo:fo + heads], exs[:])
                rc = sp.tile([128, heads], F32, tag="rc")
                nc.vector.tensor_scalar_add(rc[:], usb[:, fo:fo + heads], 1e-16)
                nc.vector.reciprocal(rc[:], rc[:])
                u = sp.tile([128, fo], F32, tag="u")
                nc.vector.tensor_tensor(
                    u[:].rearrange("p (h d) -> p h d", d=HID),
                    usb[:, 0:fo].rearrange("p (h d) -> p h d", d=HID),
                    rc[:].unsqueeze(2).broadcast_to([128, heads, HID]),
                    OP.mult)
                if cfg["elu"]:
                    nc.vector.tensor_add(u[:], u[:], brep_sb[:])
                    r1 = sp.tile([128, fo], F32, tag="r1")
                    nc.scalar.activation(r1[:], u[:], AF.Relu)
                    r2 = sp.tile([128, fo], F32, tag="r2")
                    nc.scalar.activation(r2[:], u[:], AF.Relu, scale=-1.0)
                    e2 = sp.tile([128, fo], F32, tag="e2")
                    nc.scalar.activation(e2[:], r2[:], AF.Exp, scale=-1.0)
                    nc.vector.scalar_tensor_tensor(u[:], e2[:], -1.0, r1[:],
                                                   OP.add, OP.add)
                if cfg.get("hout") is not None:
                    nc.sync.dma_start(cfg["hout"][w * 128:(w + 1) * 128, :], u[:])
                if pool_ps is not None:
                    nc.tensor.matmul(pool_ps[:], u[:], ppool_sb[:, w, :],
                                     start=(w == 0), stop=(w == NW - 1))

        sweep(lo=True)
        sweep(lo=False)

        if pool_ps is not None:
            po = sp.tile([HID, G], F32, tag="po")
            nc.scalar.copy(po[:], pool_ps[:])
            nc.sync.dma_start(cfg["poolout"][:], po[:])


def _build_layer_program(cfg):
    nc = bacc.Bacc(None, target_bir_lowering=False, num_swdge_queues=4)
    d = {}
    d["xT"] = nc.declare_dram_parameter("xT", [128, NPAD], BF16, isOutput=False)
    d["xTloc"] = nc.declare_dram_parameter("xTloc", [128, SH], BF16, isOutput=False)
    wcols = cfg["fo"] + 2 * cfg["heads"]
    d["wext"] = nc.declare_dram_parameter("wext", [128, wcols], BF16, isOutput=False)
    d["brep"] = nc.declare_dram_parameter("brep", [128, cfg["fo"]], F32, isOutput=False)
    d["srcidx"] = nc.declare_dram_parameter("srcidx", [128, cfg["EP"] // 16], I16, isOutput=False)
    d["tbl_lo"] = nc.dram_tensor("tbl_lo", [NLO, cfg["rowlen"]], cfg["tdtype"])
    d["tbl_hi"] = nc.dram_tensor("tbl_hi", [NHI, cfg["rowlen"]], cfg["tdtype"])
    d["p01"] = nc.declare_dram_parameter("p01", [128, cfg["CT"], 256], F8, isOutput=False)
    if cfg["pool"]:
        d["ppool"] = nc.declare_dram_parameter("ppool", [128, NW, G], F32, isOutput=False)
        d["poolout"] = nc.declare_dram_parameter("poolout", [HID, G], F32, isOutput=True)
    else:
        d["hout"] = nc.declare_dram_parameter("hout", [SH, cfg["fo"]], F32, isOutput=True)
    cfg = dict(cfg, **d)
    with tile.TileContext(nc) as tc:
        _emit_layer(nc, tc, cfg)
    nc.compile()
    return nc


def _build_head_program():
    nc = bacc.Bacc(None, target_bir_lowering=False)
    poolT = nc.declare_dram_parameter("poolT", [HID, G], F32, isOutput=False)
    l1w = nc.declare_dram_parameter("l1w", [HID, HID // 2], F32, isOutput=False)
    l1b = nc.declare_dram_parameter("l1b", [HID // 2, 1], F32, isOutput=False)
    l1bn = nc.declare_dram_parameter("l1bn", [HID // 2, 1], F32, isOutput=False)
    l2w = nc.declare_dram_parameter("l2w", [HID // 2, 1], F32, isOutput=False)
    l2b = nc.declare_dram_parameter("l2b", [1, 1], F32, isOutput=False)
    outT = nc.declare_dram_parameter("outT", [1, G], F32, isOutput=True)
    with tile.TileContext(nc) as tc:
        with (
            tc.tile_pool(name="h", bufs=1) as hp,
            tc.tile_pool(name="hp", bufs=2, space="PSUM") as pp,
        ):
            pt = hp.tile([HID, G], F32)
            nc.sync.dma_start(pt[:], poolT[:])
            w1 = hp.tile([HID, HID // 2], F32)
            nc.sync.dma_start(w1[:], l1w[:])
            b1 = hp.tile([HID // 2, 1], F32)
            nc.sync.dma_start(b1[:], l1b[:])
            b1n = hp.tile([HID // 2, 1], F32)
            nc.sync.dma_start(b1n[:], l1bn[:])
            w2 = hp.tile([HID // 2, 1], F32)
            nc.sync.dma_start(w2[:], l2w[:])
            b2 = hp.tile([1, 1], F32)
            nc.sync.dma_start(b2[:], l2b[:])

            ps1 = pp.tile([HID // 2, G], F32)
            nc.tensor.matmul(ps1[:], w1[:], pt[:], start=True, stop=True)
            r1 = hp.tile([HID // 2, G], F32)
            nc.scalar.activation(r1[:], ps1[:], AF.Relu, bias=b1[:, 0:1])
            r2 = hp.tile([HID // 2, G], F32)
            nc.scalar.activation(r2[:], ps1[:], AF.Relu, scale=-1.0, bias=b1n[:, 0:1])
            e2 = hp.tile([HID // 2, G], F32)
            nc.scalar.activation(e2[:], r2[:], AF.Exp, scale=-1.0)
            h = hp.tile([HID // 2, G], F32)
            nc.vector.tensor_add(h[:], r1[:], e2[:])
            nc.vector.tensor_scalar_add(h[:], h[:], -1.0)
            ps2 = pp.tile([1, G], F32)
            nc.tensor.matmul(ps2[:], w2[:], h[:], start=True, stop=True)
            o = hp.tile([1, G], F32)
            nc.scalar.activation(o[:], ps2[:], AF.Identity, bias=b2[0:1, 0:1])
            nc.sync.dma_start(outT[:], o[:])
    nc.compile()
    return nc


# ----------------------------------------------------------------------------
# entry point
# ----------------------------------------------------------------------------

def _run(nc, in_maps, core_ids, trace=False):
    import os
    tr = trace or bool(os.environ.get("KERNEL_PROFILE"))
    res = run_bass_kernel_spmd(nc, in_maps, core_ids, trace=tr)
    if res.exec_time_ns is not None:
        last_stats.setdefault("exec_ns", []).append(res.exec_time_ns)
    return res


def kernel(x, edge_index, batch, W1, att_src1, att_dst1, b1, W2, att_src2,
           att_dst2, b2, lin1_w, lin1_b, lin2_w, lin2_b):
    x = np.asarray(x, np.float32)
    per_core, CL, CH, CT, EP, maxcnt = _build_edges(edge_index)
    batch = np.asarray(batch).astype(np.int64)

    bf = ml_dtypes.bfloat16
    w1ext = _fuse_w(np.asarray(W1, np.float32),
                    np.asarray(att_src1, np.float32), np.asarray(att_dst1, np.float32),
                    H1, HID)
    w2ext = _fuse_w(np.asarray(W2, np.float32),
                    np.asarray(att_src2, np.float32), np.asarray(att_dst2, np.float32),
                    1, HID)
    b1rep = np.tile(np.asarray(b1, np.float32), (128, 1)).copy()
    b2rep = np.tile(np.asarray(b2, np.float32), (128, 1)).copy()

    xp = np.zeros((NPAD, FIN), np.float32)
    xp[:N] = x
    xT = np.ascontiguousarray(xp.T.astype(bf))

    base_cfg = dict(CL=CL, CH=CH, CT=CT, EP=EP, maxcnt=maxcnt)
    cfg1 = dict(base_cfg, tag="a", fo=FIN, heads=H1, rowlen=256, tdtype=F8,
                elu=True, pool=False)
    cfg2 = dict(base_cfg, tag="b", fo=HID, heads=1, rowlen=128, tdtype=F16,
                elu=False, pool=True)

    nc1 = _build_layer_program(cfg1)
    in_maps = []
    for c in range(NCORES):
        in_maps.append(dict(
            xT=xT, xTloc=np.ascontiguousarray(xT[:, c * SH:(c + 1) * SH]),
            wext=w1ext.astype(bf), brep=b1rep, **per_core[c]))
    r1 = _run(nc1, in_maps, list(range(NCORES)))

    h1 = np.zeros((NPAD, FIN), np.float32)
    for c in range(NCORES):
        lo, hi = c * SH, min((c + 1) * SH, N)
        h1[lo:hi] = r1.results[c]["hout"][:hi - lo]
    h1T = np.ascontiguousarray(h1.T.astype(bf))

    # pooling matrices with 1/count folded in
    counts = np.bincount(batch, minlength=G).astype(np.float32)
    recip = 1.0 / np.maximum(counts, 1.0)
    nc2 = _build_layer_program(cfg2)
    in_maps2 = []
    for c in range(NCORES):
        pb = np.zeros((SH, G), np.float32)
        lo, hi = c * SH, min((c + 1) * SH, N)
        if hi > lo:
            rows = np.arange(hi - lo)
            pb[rows, batch[lo:hi]] = recip[batch[lo:hi]]
        ppool = np.ascontiguousarray(pb.reshape(NW, 128, G).transpose(1, 0, 2))
        in_maps2.append(dict(
            xT=h1T, xTloc=np.ascontiguousarray(h1T[:, c * SH:(c + 1) * SH]),
            wext=w2ext.astype(bf), brep=b2rep, ppool=ppool, **per_core[c]))
    r2 = _run(nc2, in_maps2, list(range(NCORES)))

    poolT = np.zeros((HID, G), np.float32)
    for c in range(NCORES):
        poolT += r2.results[c]["poolout"]
    # layer-2 bias is linear through the mean-pool: add it here instead of
    # per-node on device
    poolT += np.asarray(b2, np.float32).reshape(HID, 1)

    nc3 = _build_head_program()
    l1b_ = np.asarray(lin1_b, np.float32).reshape(HID // 2, 1)
    r3 = _run(nc3, [dict(
        poolT=poolT, l1w=np.asarray(lin1_w, np.float32), l1b=l1b_, l1bn=-l1b_,
        l2w=np.asarray(lin2_w, np.float32),
        l2b=np.asarray(lin2_b, np.float32).reshape(1, 1))], [0])
    return np.ascontiguousarray(r3.results[0]["outT"].reshape(G, 1))
